# revision 21
# baseline (speedup 1.0000x reference)
"""Trainium2 Bass kernel for nn_BbVertLoss (point-in-bbox CE + IoU + L2 loss).

Strategy v2 (hardcoded for B=16, N=40960, H=24, 8 cores):
  - Shard H across cores: core k handles h in {3k, 3k+1, 3k+2} for ALL 16
    batches and all points. Partition row = b*8 + blk holds 5120 consecutive
    points of batch b, processed in 2 column-chunks of 2560 for SBUF fit and
    DMA/compute overlap. Free dim per instruction is 2560 (vs 640 in v1),
    cutting the ~150-350 cycle fixed per-instruction overhead from ~35% to
    ~10%, and cutting READ_ACCUM count from 96 to 24 per core.
  - Math per (batch, box h, point), identical to v1:
      u_d   = (a_d - x_d)(x_d - b_d) = r_d^2 - (x_d - c_d)^2,  c=(a+b)/2
      pred: p = sigmoid(-100*clip(w, -0.2, 0.2)),  w = max_d((x_d-c_d)^2-r_d^2)
      gt:   g = 1{w' < 0}
      ce    = -log(|p + (g-1)| + eps)
      TP    = (sum|p+(g-1)| + sum p + sum g - Npts) / 2
    Per-(partition, h, chunk) sums S_p, S_g, S_sel, S_ln via accum_out.
  - ACT table sets: Square lives in every set, so only Sigmoid/Ln loads
    occur; phases are ordered so sq runs ride in whatever set is loaded.
  - Host: partition+chunk+core reduction and final combine in f64.
"""

import numpy as np

B, N, H = 16, 40960, 24
NCORES = 8
HPC = H // NCORES            # h per core = 3
BLK = 8                      # partition blocks per batch
NPART = B * BLK              # 128
FPT = N // BLK               # points per partition = 5120
NCH = 2                      # column chunks
CWS = [3072, 2048]           # uneven: big first (DMA overlap), small last
                             # (shorter serial sigmoid->sel->ln tail)
SCW = 16                     # scal columns per h

_CACHE = {}


def _register_custom_ops():
    """Register fused DVE ops in the module-level registries (idempotent)."""
    import concourse.dve_ops as dops
    from concourse.dve_spec import (Spec, Src0, Src1, C0, C1, C2, Zero, One,
                                    maxx, minn, sq, lower, AluOp)
    from concourse.dve_table_gen import dve_ver_for
    from concourse.dve_uop import DveOpSpec

    if "ANT_SUB2MAX" in dops._SUB_OPCODE_FOR_NAME:
        _CACHE["ops"] = {o.name: o for o in dops.OPS}
        return

    ver = dve_ver_for("TRN2")

    def ref_sub2max(in0, in1, s0, s1, imm2):
        return np.maximum(in0 - s0, in1 - s1)

    def ref_sqmaxclip(in0, in1, s0, s1, imm2):
        return np.minimum(np.maximum(np.maximum((in0 - s0) ** 2 - s1, in1),
                                     imm2), -imm2)

    def ref_sqmaxlt0sum(in0, in1, s0, s1, imm2):
        b = (np.maximum((in0 - s0) ** 2 - s1, in1) < 0.0).astype(np.float32)
        return b, b.reshape(b.shape[0], -1).sum(axis=-1, keepdims=True).astype(
            np.float32)

    def ref_abspg1sum(in0, in1, s0, s1, imm2):
        t = ((in1 - np.float32(1.0)) + in0).astype(np.float32)
        b = np.abs(t)
        return b, b.reshape(b.shape[0], -1).sum(axis=-1, keepdims=True).astype(
            np.float32)

    def ref_sqsubmax(in0, in1, s0, s1, imm2):
        return np.maximum((in0 - s0) ** 2 - s1, in1)

    _t = Src0 + (Src1 - One)
    specs = [
        ("ANT_SUB2MAX", Spec(body=maxx(Src0 - C0, Src1 - C1),
                             reference=ref_sub2max)),
        ("ANT_SQMAXCLIP", Spec(body=minn(maxx(maxx(sq(Src0 - C0) - C1, Src1),
                                              C2), Zero - C2),
                               reference=ref_sqmaxclip)),
        ("ANT_SQMAXLT0SUM", Spec(body=(maxx(sq(Src0 - C0) - C1, Src1) < Zero),
                                 accum=AluOp.ADD, reference=ref_sqmaxlt0sum)),
        ("ANT_ABSPG1SUM", Spec(body=maxx(_t, Zero - _t),
                               accum=AluOp.ADD, reference=ref_abspg1sum)),
        ("ANT_SQSUBMAX", Spec(body=maxx(sq(Src0 - C0) - C1, Src1),
                              reference=ref_sqsubmax)),
        ("ANT_SQSUB", Spec(body=sq(Src0 - C0) - C1,
                           reference=lambda in0, in1, s0, s1, imm2:
                               (in0 - s0) ** 2 - s1)),
    ]
    for name, spec in specs:
        opcode = max(dops._SUB_OPCODE_FOR_NAME.values()) + 1
        assert opcode < 0x20
        tmp = DveOpSpec(name=name, opcode=opcode, uops=lower(spec, ver=ver),
                        rd1_en=True)
        op = dops.DveOp(name, spec, subdim=False, uops_sha={ver: tmp.sha(ver)})
        dops.OPS.append(op)
        dops.CUSTOM_DVE_SPECS[name] = spec
        dops._SUB_OPCODE_FOR_NAME[name] = opcode
    _CACHE["ops"] = {o.name: o for o in dops.OPS}


def _build_module():
    import concourse.bacc as bacc
    import concourse.tile as tile
    from concourse import mybir

    _register_custom_ops()
    OPS = _CACHE["ops"]

    f32 = mybir.dt.float32
    bf16 = mybir.dt.bfloat16
    Act = mybir.ActivationFunctionType

    nc = bacc.Bacc("TRN2", debug=False)

    xin = nc.dram_tensor("xin", [NPART, FPT], f32, kind="ExternalInput")
    yzin = nc.dram_tensor("yzin", [NPART, 2 * FPT], bf16,
                          kind="ExternalInput")
    scal = nc.dram_tensor("scal", [NPART, HPC * SCW], f32,
                          kind="ExternalInput")
    accs_d = nc.dram_tensor("accs", [NPART, 4 * HPC * NCH], f32,
                            kind="ExternalOutput")

    with tile.TileContext(nc) as tc:
        with (
            tc.tile_pool(name="data", bufs=1) as data,
            tc.tile_pool(name="chunk", bufs=2) as chunk,
            tc.tile_pool(name="sq", bufs=2) as sqp,
            tc.tile_pool(name="mid", bufs=2) as midp,
            tc.tile_pool(name="late", bufs=2) as late,
            tc.tile_pool(name="tclp", bufs=3) as tclp,
            tc.tile_pool(name="selp", bufs=3) as selp,
        ):
            eps8 = data.tile([NPART, 1], f32, tag="eps8")
            nc.vector.memset(eps8[:], 1e-8)
            # preload the sigmoid table set during the first chunk's DMA
            warm = data.tile([NPART, 1], f32, tag="warm")
            nc.scalar.activation(warm[:], eps8[:], Act.Sigmoid,
                                 bias=0.0, scale=-100.0)
            sc = data.tile([NPART, HPC * SCW], f32, tag="sc")
            nc.sync.dma_start(out=sc[:], in_=scal[:])

            # one [P, 4*HPC*NCH] accumulator tile: P | G | S | L blocks
            accs = data.tile([NPART, 4 * HPC * NCH], f32, tag="accs")
            W = HPC * NCH
            accP, accG = accs[:, 0:W], accs[:, W:2 * W]
            accS, accL = accs[:, 2 * W:3 * W], accs[:, 3 * W:4 * W]

            def col(j, i):
                return sc[:, SCW * j + i : SCW * j + i + 1]

            off = 0
            for c in range(NCH):
                CW = CWS[c]
                # DMA order y, z, x (first compute consumes y); y/z in bf16.
                rawyz = chunk.tile([NPART, 2 * CW], bf16, tag="rawyz")
                for d in (0, 1):
                    src = yzin[:, d * FPT + off : d * FPT + off + CW]
                    for q in range(2):
                        r0, r1 = 64 * q, 64 * (q + 1)
                        nc.sync.dma_start(
                            out=rawyz[r0:r1, d * CW:(d + 1) * CW],
                            in_=src[r0:r1, :])
                rawx = chunk.tile([NPART, CW], f32, tag="rawx")
                for q in range(2):
                    r0, r1 = 64 * q, 64 * (q + 1)
                    nc.sync.dma_start(out=rawx[r0:r1, :],
                                      in_=xin[r0:r1, off:off + CW])
                off += CW
                xs = [rawx[:, :], rawyz[:, 0:CW], rawyz[:, CW:2 * CW]]

                def pred_leg(j):
                    # pred leg: w = max_d((x_d-c_d)^2 - r_d^2), clip +-0.2
                    sqy = sqp.tile([NPART, CW], f32, tag="sqy")
                    nc.scalar.activation(sqy[:], xs[1], Act.Square,
                                         bias=col(j, 0), scale=1.0)
                    sqz = sqp.tile([NPART, CW], f32, tag="sqz")
                    nc.scalar.activation(sqz[:], xs[2], Act.Square,
                                         bias=col(j, 1), scale=1.0)
                    m1 = midp.tile([NPART, CW], f32, tag="m1")
                    nc.vector._custom_dve(OPS["ANT_SUB2MAX"], out=m1[:],
                                          in0=sqy[:], in1=sqz[:],
                                          s0=col(j, 2), s1=col(j, 3))
                    tcl = tclp.tile([NPART, CW], f32, tag="tcl")
                    nc.vector._custom_dve(OPS["ANT_SQMAXCLIP"], out=tcl[:],
                                          in0=xs[0], in1=m1[:],
                                          s0=col(j, 4), s1=col(j, 5),
                                          imm2=-0.2)
                    return tcl

                sels = {}
                tcl_last = pred_leg(HPC - 1) if c == NCH - 1 else None
                for j in range(HPC):
                    def acc(t):
                        return t[:, NCH * j + c : NCH * j + c + 1]

                    tcl = tcl_last if (tcl_last is not None and j == HPC - 1
                                       ) else pred_leg(j)

                    # gt leg: g = 1{max_d((x_d-c'_d)^2 - r'^2_d) < 0}
                    sqgy = sqp.tile([NPART, CW], f32, tag="sqy")
                    nc.scalar.activation(sqgy[:], xs[1], Act.Square,
                                         bias=col(j, 8), scale=1.0)
                    sqgz = sqp.tile([NPART, CW], f32, tag="sqz")
                    nc.scalar.activation(sqgz[:], xs[2], Act.Square,
                                         bias=col(j, 9), scale=1.0)
                    mg1 = midp.tile([NPART, CW], f32, tag="m1")
                    nc.vector._custom_dve(OPS["ANT_SUB2MAX"], out=mg1[:],
                                          in0=sqgy[:], in1=sqgz[:],
                                          s0=col(j, 10), s1=col(j, 11))
                    g = late.tile([NPART, CW], bf16, tag="g")
                    nc.vector._custom_dve(OPS["ANT_SQMAXLT0SUM"], out=g[:],
                                          in0=xs[0], in1=mg1[:],
                                          s0=col(j, 12), s1=col(j, 13),
                                          accum_out=acc(accG))

                    # sigmoid + sel (Square rides in the sigmoid table set)
                    p = midp.tile([NPART, CW], f32, tag="m1")
                    nc.scalar.activation(p[:], tcl[:], Act.Sigmoid,
                                         bias=0.0, scale=-100.0,
                                         accum_out=acc(accP))
                    sel = selp.tile([NPART, CW], bf16, tag="sel")
                    nc.vector._custom_dve(OPS["ANT_ABSPG1SUM"], out=sel[:],
                                          in0=p[:], in1=g[:],
                                          accum_out=acc(accS))
                    sels[j] = sel
                for j in range(HPC):   # Ln phase (one table load)
                    lnsel = midp.tile([NPART, CW], f32, tag="m1")
                    nc.scalar.activation(lnsel[:], sels[j][:], Act.Ln,
                                         bias=eps8[:], scale=1.0,
                                         accum_out=accL[:, NCH * j + c :
                                                        NCH * j + c + 1])

            nc.sync.dma_start(out=accs_d[:], in_=accs[:])

    nc.compile()
    return nc


def _get_module():
    if "nc" not in _CACHE:
        _CACHE["nc"] = _build_module()
    return _CACHE["nc"]


def _make_inputs(X_pc, y_bbvert_pred, Y_bbvert):
    """Build per-core input maps (host-side layout + scalar precompute)."""
    X_pc = np.asarray(X_pc, dtype=np.float32)
    pred = np.asarray(y_bbvert_pred, dtype=np.float32)
    gt = np.asarray(Y_bbvert, dtype=np.float32)

    # layout: row = b*8 + blk; x as f32 [128, 5120]; y|z as bf16
    # [128, 2*5120]; identical for every core.
    import ml_dtypes
    xyz = X_pc[:, :, :3].reshape(B, BLK, FPT, 3).transpose(0, 1, 3, 2)
    xyz = xyz.reshape(NPART, 3, FPT)
    xin = np.ascontiguousarray(xyz[:, 0, :])
    yzin = np.ascontiguousarray(
        xyz[:, 1:3, :].reshape(NPART, 2 * FPT)).astype(ml_dtypes.bfloat16)

    # columns per (B,H): pred [-c_y, -c_z, rsq_y, rsq_z, c_x, rsq_x, c_y, c_z]
    # then gt [-c'_y, -c'_z, rsq'_y, rsq'_z, c'_x, rsq'_x], 2 pad
    def params(t, with_pos):
        a = t[:, :, 0, :]
        b = t[:, :, 1, :]
        c = ((a + b) * np.float32(0.5)).astype(np.float32)
        r = ((b - a) * np.float32(0.5)).astype(np.float32)
        rsq = (r * r).astype(np.float32)
        cols = [-c[:, :, 1], -c[:, :, 2], rsq[:, :, 1], rsq[:, :, 2],
                c[:, :, 0], rsq[:, :, 0]]
        if with_pos:
            cols += [c[:, :, 1], c[:, :, 2]]
        return np.stack(cols, axis=-1)

    zpad = np.zeros((B, H, 2), dtype=np.float32)
    sc_all = np.concatenate([params(pred, True), params(gt, False), zpad],
                            axis=-1)  # [B,H,16]

    in_maps = []
    for k in range(NCORES):
        hsel = sc_all[:, HPC * k : HPC * (k + 1), :]        # [B,HPC,16]
        scs = np.repeat(hsel, BLK, axis=0)                  # [128,HPC,16]
        in_maps.append({
            "xin": xin,
            "yzin": yzin,
            "scal": np.ascontiguousarray(scs.reshape(NPART, HPC * SCW)),
        })
    return in_maps


def _combine(results, y_bbvert_pred, Y_bbvert):
    """Host-side: partition+chunk+core reduction and final combine (f64)."""
    pred = np.asarray(y_bbvert_pred, dtype=np.float32)
    gt = np.asarray(Y_bbvert, dtype=np.float32)

    Sp = np.zeros((B, H)); Sg = np.zeros((B, H))
    Ss = np.zeros((B, H)); Sl = np.zeros((B, H))
    W = HPC * NCH
    for k in range(NCORES):
        r = results[k]["accs"].astype(np.float64)
        for i, S in enumerate((Sp, Sg, Ss, Sl)):
            a = r[:, i * W:(i + 1) * W].reshape(B, BLK, HPC, NCH)
            S[:, HPC * k : HPC * (k + 1)] = a.sum(axis=(1, 3))

    Tp = (Ss + Sg + Sp - float(N)) * 0.5
    helper = (gt.reshape(B, H, 6).sum(axis=-1) > 0.0).astype(np.float64)
    Sce = -Sl

    denom_ce = helper.sum() * N
    loss_ce = (Sce * helper).sum() / denom_ce

    iou_all = -(Tp / (Sp + Sg - Tp + 1e-6))
    loss_iou = (iou_all * helper).sum() / helper.sum()

    l2_all = ((gt.astype(np.float64) - pred.astype(np.float64)) ** 2
              ).reshape(B, H, 6).mean(axis=-1)
    l2_pos = (l2_all * helper).sum() / helper.sum()
    negw = (1.0 - helper)[:, :, None]
    dneg = (pred[:, :, 0, :].astype(np.float64)
            - pred[:, :, 1, :].astype(np.float64))
    l2_neg = ((negw * dneg) ** 2).sum() / ((1.0 - helper).sum() + 1e-8)
    loss_l2 = l2_pos + l2_neg

    total = loss_ce + loss_l2 + loss_iou
    return (np.float32(total), np.float32(loss_l2),
            np.float32(loss_ce), np.float32(loss_iou))


def run(X_pc, y_bbvert_pred, Y_bbvert, trace=False):
    from concourse.bass_utils import run_bass_kernel_spmd

    nc = _get_module()
    in_maps = _make_inputs(X_pc, y_bbvert_pred, Y_bbvert)
    res = run_bass_kernel_spmd(nc, in_maps, core_ids=list(range(NCORES)),
                               trace=trace)
    out = _combine(res.results, y_bbvert_pred, Y_bbvert)
    return out, res


def kernel(X_pc, y_bbvert_pred, Y_bbvert):
    out, _ = run(X_pc, y_bbvert_pred, Y_bbvert, trace=False)
    return out


# revision 22
# speedup vs baseline: 1.0657x; 1.0657x over previous
"""Trainium2 Bass kernel for nn_BbVertLoss (point-in-bbox CE + IoU + L2 loss).

Strategy v2 (hardcoded for B=16, N=40960, H=24, 8 cores):
  - Shard H across cores: core k handles h in {3k, 3k+1, 3k+2} for ALL 16
    batches and all points. Partition row = b*8 + blk holds 5120 consecutive
    points of batch b, processed in 2 column-chunks of 2560 for SBUF fit and
    DMA/compute overlap. Free dim per instruction is 2560 (vs 640 in v1),
    cutting the ~150-350 cycle fixed per-instruction overhead from ~35% to
    ~10%, and cutting READ_ACCUM count from 96 to 24 per core.
  - Math per (batch, box h, point), identical to v1:
      u_d   = (a_d - x_d)(x_d - b_d) = r_d^2 - (x_d - c_d)^2,  c=(a+b)/2
      pred: p = sigmoid(-100*clip(w, -0.2, 0.2)),  w = max_d((x_d-c_d)^2-r_d^2)
      gt:   g = 1{w' < 0}
      ce    = -log(|p + (g-1)| + eps)
      TP    = (sum|p+(g-1)| + sum p + sum g - Npts) / 2
    Per-(partition, h, chunk) sums S_p, S_g, S_sel, S_ln via accum_out.
  - ACT table sets: Square lives in every set, so only Sigmoid/Ln loads
    occur; phases are ordered so sq runs ride in whatever set is loaded.
  - Host: partition+chunk+core reduction and final combine in f64.
"""

import numpy as np

B, N, H = 16, 40960, 24
NCORES = 8
HPC = H // NCORES            # h per core = 3
BLK = 8                      # partition blocks per batch
NPART = B * BLK              # 128
FPT = N // BLK               # points per partition = 5120
NCH = 2                      # column chunks
CWS = [3072, 2048]           # uneven: big first (DMA overlap), small last
                             # (shorter serial sigmoid->sel->ln tail)
SCW = 16                     # scal columns per h

_CACHE = {}


def _register_custom_ops():
    """Register fused DVE ops in the module-level registries (idempotent)."""
    import concourse.dve_ops as dops
    from concourse.dve_spec import (Spec, Src0, Src1, C0, C1, C2, Zero, One,
                                    maxx, minn, sq, lower, AluOp)
    from concourse.dve_table_gen import dve_ver_for
    from concourse.dve_uop import DveOpSpec

    if "ANT_SUB2MAX" in dops._SUB_OPCODE_FOR_NAME:
        _CACHE["ops"] = {o.name: o for o in dops.OPS}
        return

    ver = dve_ver_for("TRN2")

    def ref_sub2max(in0, in1, s0, s1, imm2):
        return np.maximum(in0 - s0, in1 - s1)

    def ref_sqmaxclip(in0, in1, s0, s1, imm2):
        return np.minimum(np.maximum(np.maximum((in0 - s0) ** 2 - s1, in1),
                                     imm2), -imm2)

    def ref_sqmaxlt0sum(in0, in1, s0, s1, imm2):
        b = (np.maximum((in0 - s0) ** 2 - s1, in1) < 0.0).astype(np.float32)
        return b, b.reshape(b.shape[0], -1).sum(axis=-1, keepdims=True).astype(
            np.float32)

    def ref_abspg1sum(in0, in1, s0, s1, imm2):
        t = ((in1 - np.float32(1.0)) + in0).astype(np.float32)
        b = np.abs(t)
        return b, b.reshape(b.shape[0], -1).sum(axis=-1, keepdims=True).astype(
            np.float32)

    def ref_sqsubmax(in0, in1, s0, s1, imm2):
        return np.maximum((in0 - s0) ** 2 - s1, in1)

    _t = Src0 + (Src1 - One)
    specs = [
        ("ANT_SUB2MAX", Spec(body=maxx(Src0 - C0, Src1 - C1),
                             reference=ref_sub2max)),
        ("ANT_SQMAXCLIP", Spec(body=minn(maxx(maxx(sq(Src0 - C0) - C1, Src1),
                                              C2), Zero - C2),
                               reference=ref_sqmaxclip)),
        ("ANT_SQMAXLT0SUM", Spec(body=(maxx(sq(Src0 - C0) - C1, Src1) < Zero),
                                 accum=AluOp.ADD, reference=ref_sqmaxlt0sum)),
        ("ANT_ABSPG1SUM", Spec(body=maxx(_t, Zero - _t),
                               accum=AluOp.ADD, reference=ref_abspg1sum)),
        ("ANT_SQSUBMAX", Spec(body=maxx(sq(Src0 - C0) - C1, Src1),
                              reference=ref_sqsubmax)),
        ("ANT_SQSUB", Spec(body=sq(Src0 - C0) - C1,
                           reference=lambda in0, in1, s0, s1, imm2:
                               (in0 - s0) ** 2 - s1)),
    ]
    for name, spec in specs:
        opcode = max(dops._SUB_OPCODE_FOR_NAME.values()) + 1
        assert opcode < 0x20
        tmp = DveOpSpec(name=name, opcode=opcode, uops=lower(spec, ver=ver),
                        rd1_en=True)
        op = dops.DveOp(name, spec, subdim=False, uops_sha={ver: tmp.sha(ver)})
        dops.OPS.append(op)
        dops.CUSTOM_DVE_SPECS[name] = spec
        dops._SUB_OPCODE_FOR_NAME[name] = opcode
    _CACHE["ops"] = {o.name: o for o in dops.OPS}


def _build_module():
    import concourse.bacc as bacc
    import concourse.tile as tile
    from concourse import mybir

    _register_custom_ops()
    OPS = _CACHE["ops"]

    f32 = mybir.dt.float32
    bf16 = mybir.dt.bfloat16
    Act = mybir.ActivationFunctionType

    nc = bacc.Bacc("TRN2", debug=False)

    xin = nc.dram_tensor("xin", [NPART, FPT], f32, kind="ExternalInput")
    yzin = nc.dram_tensor("yzin", [NPART, 2 * FPT], bf16,
                          kind="ExternalInput")
    scal = nc.dram_tensor("scal", [NPART, HPC * SCW], f32,
                          kind="ExternalInput")
    accs_d = nc.dram_tensor("accs", [NPART, 4 * HPC * NCH], f32,
                            kind="ExternalOutput")

    with tile.TileContext(nc) as tc:
        with (
            tc.tile_pool(name="data", bufs=1) as data,
            tc.tile_pool(name="chunk", bufs=2) as chunk,
            tc.tile_pool(name="sq", bufs=2) as sqp,
            tc.tile_pool(name="mid", bufs=2) as midp,
            tc.tile_pool(name="late", bufs=2) as late,
            tc.tile_pool(name="tclp", bufs=3) as tclp,
            tc.tile_pool(name="selp", bufs=3) as selp,
        ):
            eps8 = data.tile([NPART, 1], f32, tag="eps8")
            nc.vector.memset(eps8[:], 1e-8)
            # preload the sigmoid table set during the first chunk's DMA
            warm = data.tile([NPART, 1], f32, tag="warm")
            nc.scalar.activation(warm[:], eps8[:], Act.Sigmoid,
                                 bias=0.0, scale=-100.0)
            sc = data.tile([NPART, HPC * SCW], f32, tag="sc")
            nc.sync.dma_start(out=sc[:], in_=scal[:])

            # one [P, 4*HPC*NCH] accumulator tile: P | G | S | L blocks
            accs = data.tile([NPART, 4 * HPC * NCH], f32, tag="accs")
            W = HPC * NCH
            accP, accG = accs[:, 0:W], accs[:, W:2 * W]
            accS, accL = accs[:, 2 * W:3 * W], accs[:, 3 * W:4 * W]

            def col(j, i):
                return sc[:, SCW * j + i : SCW * j + i + 1]

            off = 0
            for c in range(NCH):
                CW = CWS[c]
                # DMA order y, z, x (first compute consumes y); y/z in bf16.
                rawyz = chunk.tile([NPART, 2 * CW], bf16, tag="rawyz")
                for d in (0, 1):
                    src = yzin[:, d * FPT + off : d * FPT + off + CW]
                    for q in range(2):
                        r0, r1 = 64 * q, 64 * (q + 1)
                        nc.sync.dma_start(
                            out=rawyz[r0:r1, d * CW:(d + 1) * CW],
                            in_=src[r0:r1, :])
                rawx = chunk.tile([NPART, CW], f32, tag="rawx")
                for q in range(2):
                    r0, r1 = 64 * q, 64 * (q + 1)
                    nc.sync.dma_start(out=rawx[r0:r1, :],
                                      in_=xin[r0:r1, off:off + CW])
                off += CW
                xs = [rawx[:, :], rawyz[:, 0:CW], rawyz[:, CW:2 * CW]]

                def pred_leg(j):
                    # pred leg: w = max_d((x_d-c_d)^2 - r_d^2), clip +-0.2
                    sqy = sqp.tile([NPART, CW], bf16, tag="sqy")
                    nc.scalar.activation(sqy[:], xs[1], Act.Square,
                                         bias=col(j, 0), scale=1.0)
                    sqz = sqp.tile([NPART, CW], bf16, tag="sqz")
                    nc.scalar.activation(sqz[:], xs[2], Act.Square,
                                         bias=col(j, 1), scale=1.0)
                    m1 = midp.tile([NPART, CW], f32, tag="m1")
                    nc.vector._custom_dve(OPS["ANT_SUB2MAX"], out=m1[:],
                                          in0=sqy[:], in1=sqz[:],
                                          s0=col(j, 2), s1=col(j, 3))
                    tcl = tclp.tile([NPART, CW], f32, tag="tcl")
                    nc.vector._custom_dve(OPS["ANT_SQMAXCLIP"], out=tcl[:],
                                          in0=xs[0], in1=m1[:],
                                          s0=col(j, 4), s1=col(j, 5),
                                          imm2=-0.2)
                    return tcl

                sels = {}
                tcl_last = pred_leg(HPC - 1) if c == NCH - 1 else None
                for j in range(HPC):
                    def acc(t):
                        return t[:, NCH * j + c : NCH * j + c + 1]

                    tcl = tcl_last if (tcl_last is not None and j == HPC - 1
                                       ) else pred_leg(j)

                    # gt leg: g = 1{max_d((x_d-c'_d)^2 - r'^2_d) < 0}
                    sqgy = sqp.tile([NPART, CW], bf16, tag="sqy")
                    nc.scalar.activation(sqgy[:], xs[1], Act.Square,
                                         bias=col(j, 8), scale=1.0)
                    sqgz = sqp.tile([NPART, CW], bf16, tag="sqz")
                    nc.scalar.activation(sqgz[:], xs[2], Act.Square,
                                         bias=col(j, 9), scale=1.0)
                    mg1 = midp.tile([NPART, CW], f32, tag="m1")
                    nc.vector._custom_dve(OPS["ANT_SUB2MAX"], out=mg1[:],
                                          in0=sqgy[:], in1=sqgz[:],
                                          s0=col(j, 10), s1=col(j, 11))
                    g = late.tile([NPART, CW], bf16, tag="g")
                    nc.vector._custom_dve(OPS["ANT_SQMAXLT0SUM"], out=g[:],
                                          in0=xs[0], in1=mg1[:],
                                          s0=col(j, 12), s1=col(j, 13),
                                          accum_out=acc(accG))

                    # sigmoid + sel (Square rides in the sigmoid table set)
                    p = late.tile([NPART, CW], f32, tag="p")
                    nc.scalar.activation(p[:], tcl[:], Act.Sigmoid,
                                         bias=0.0, scale=-100.0,
                                         accum_out=acc(accP))
                    sel = selp.tile([NPART, CW], bf16, tag="sel")
                    nc.vector._custom_dve(OPS["ANT_ABSPG1SUM"], out=sel[:],
                                          in0=p[:], in1=g[:],
                                          accum_out=acc(accS))
                    sels[j] = sel
                for j in range(HPC):   # Ln phase (one table load)
                    lnsel = midp.tile([NPART, CW], f32, tag="m1")
                    nc.scalar.activation(lnsel[:], sels[j][:], Act.Ln,
                                         bias=eps8[:], scale=1.0,
                                         accum_out=accL[:, NCH * j + c :
                                                        NCH * j + c + 1])

            nc.sync.dma_start(out=accs_d[:], in_=accs[:])

    nc.compile()
    return nc


def _get_module():
    if "nc" not in _CACHE:
        _CACHE["nc"] = _build_module()
    return _CACHE["nc"]


def _make_inputs(X_pc, y_bbvert_pred, Y_bbvert):
    """Build per-core input maps (host-side layout + scalar precompute)."""
    X_pc = np.asarray(X_pc, dtype=np.float32)
    pred = np.asarray(y_bbvert_pred, dtype=np.float32)
    gt = np.asarray(Y_bbvert, dtype=np.float32)

    # layout: row = b*8 + blk; x as f32 [128, 5120]; y|z as bf16
    # [128, 2*5120]; identical for every core.
    import ml_dtypes
    xyz = X_pc[:, :, :3].reshape(B, BLK, FPT, 3).transpose(0, 1, 3, 2)
    xyz = xyz.reshape(NPART, 3, FPT)
    xin = np.ascontiguousarray(xyz[:, 0, :])
    yzin = np.ascontiguousarray(
        xyz[:, 1:3, :].reshape(NPART, 2 * FPT)).astype(ml_dtypes.bfloat16)

    # columns per (B,H): pred [-c_y, -c_z, rsq_y, rsq_z, c_x, rsq_x, c_y, c_z]
    # then gt [-c'_y, -c'_z, rsq'_y, rsq'_z, c'_x, rsq'_x], 2 pad
    def params(t, with_pos):
        a = t[:, :, 0, :]
        b = t[:, :, 1, :]
        c = ((a + b) * np.float32(0.5)).astype(np.float32)
        r = ((b - a) * np.float32(0.5)).astype(np.float32)
        rsq = (r * r).astype(np.float32)
        cols = [-c[:, :, 1], -c[:, :, 2], rsq[:, :, 1], rsq[:, :, 2],
                c[:, :, 0], rsq[:, :, 0]]
        if with_pos:
            cols += [c[:, :, 1], c[:, :, 2]]
        return np.stack(cols, axis=-1)

    zpad = np.zeros((B, H, 2), dtype=np.float32)
    sc_all = np.concatenate([params(pred, True), params(gt, False), zpad],
                            axis=-1)  # [B,H,16]

    in_maps = []
    for k in range(NCORES):
        hsel = sc_all[:, HPC * k : HPC * (k + 1), :]        # [B,HPC,16]
        scs = np.repeat(hsel, BLK, axis=0)                  # [128,HPC,16]
        in_maps.append({
            "xin": xin,
            "yzin": yzin,
            "scal": np.ascontiguousarray(scs.reshape(NPART, HPC * SCW)),
        })
    return in_maps


def _combine(results, y_bbvert_pred, Y_bbvert):
    """Host-side: partition+chunk+core reduction and final combine (f64)."""
    pred = np.asarray(y_bbvert_pred, dtype=np.float32)
    gt = np.asarray(Y_bbvert, dtype=np.float32)

    Sp = np.zeros((B, H)); Sg = np.zeros((B, H))
    Ss = np.zeros((B, H)); Sl = np.zeros((B, H))
    W = HPC * NCH
    for k in range(NCORES):
        r = results[k]["accs"].astype(np.float64)
        for i, S in enumerate((Sp, Sg, Ss, Sl)):
            a = r[:, i * W:(i + 1) * W].reshape(B, BLK, HPC, NCH)
            S[:, HPC * k : HPC * (k + 1)] = a.sum(axis=(1, 3))

    Tp = (Ss + Sg + Sp - float(N)) * 0.5
    helper = (gt.reshape(B, H, 6).sum(axis=-1) > 0.0).astype(np.float64)
    Sce = -Sl

    denom_ce = helper.sum() * N
    loss_ce = (Sce * helper).sum() / denom_ce

    iou_all = -(Tp / (Sp + Sg - Tp + 1e-6))
    loss_iou = (iou_all * helper).sum() / helper.sum()

    l2_all = ((gt.astype(np.float64) - pred.astype(np.float64)) ** 2
              ).reshape(B, H, 6).mean(axis=-1)
    l2_pos = (l2_all * helper).sum() / helper.sum()
    negw = (1.0 - helper)[:, :, None]
    dneg = (pred[:, :, 0, :].astype(np.float64)
            - pred[:, :, 1, :].astype(np.float64))
    l2_neg = ((negw * dneg) ** 2).sum() / ((1.0 - helper).sum() + 1e-8)
    loss_l2 = l2_pos + l2_neg

    total = loss_ce + loss_l2 + loss_iou
    return (np.float32(total), np.float32(loss_l2),
            np.float32(loss_ce), np.float32(loss_iou))


def run(X_pc, y_bbvert_pred, Y_bbvert, trace=False):
    from concourse.bass_utils import run_bass_kernel_spmd

    nc = _get_module()
    in_maps = _make_inputs(X_pc, y_bbvert_pred, Y_bbvert)
    res = run_bass_kernel_spmd(nc, in_maps, core_ids=list(range(NCORES)),
                               trace=trace)
    out = _combine(res.results, y_bbvert_pred, Y_bbvert)
    return out, res


def kernel(X_pc, y_bbvert_pred, Y_bbvert):
    out, _ = run(X_pc, y_bbvert_pred, Y_bbvert, trace=False)
    return out
